# revision 16
# baseline (speedup 1.0000x reference)
"""Trainium2 Bass kernel for nn_Block_19524921327813 (moe_routing).

Mixture-of-depths block: router top-k (CAP=1024 of S=2048) -> gathered
q path (qln, q-proj, rope) + full-seq kv path (vln, kv-proj, rope) ->
MHA -> out-proj + SwiGLU FFN -> weighted scatter-add into seq.

Sharding: 8 cores = 4 batches x 2 query-halves. Each core runs the full
kv pipeline for its batch (duplicated within the pair) and 512 of the
1024 selected query tokens. No cross-core communication; the host
slices inputs and scatter-adds the per-core updates into the output.

v2: bf16 activations, fp8e4m3 DoubleRow matmuls on the K-heavy
projections, resident fp8 weights loaded in a handful of large DMAs,
partition-aligned stats (no row-move DMAs), DMA-free attention
epilogue, bf16 output.
"""
import numpy as np
import ml_dtypes

import concourse.bacc as bacc
import concourse.mybir as mybir
import concourse.tile as tile
from concourse.bass_utils import run_bass_kernel_spmd
from concourse.alu_op_type import AluOpType

F32 = mybir.dt.float32
F32R = mybir.dt.float32r
BF16 = mybir.dt.bfloat16
F8 = mybir.dt.float8e4
I16 = mybir.dt.int16
U8 = mybir.dt.uint8
U32 = mybir.dt.uint32
AF = mybir.ActivationFunctionType
OP = AluOpType
AX = mybir.AxisListType
DR = mybir.MatmulPerfMode.DoubleRow

NP_F8 = ml_dtypes.float8_e4m3
NP_BF16 = ml_dtypes.bfloat16

B, S, D, H, HD = 4, 2048, 1024, 16, 64
CAP = S // 2          # 1024 selected tokens per batch
NQ = CAP // 2         # 512 query tokens per core
ROPE_BASE = 10000.0
LN_EPS = 1e-5
NCH = D // 128        # 8 d-chunks
NPR = NCH // 2        # 4 d-chunk pairs (DoubleRow)
NSCH = S // 128       # 16 key chunks
NJCH = 8              # 2-head blocks
VE = HD + 2           # PV out rows per head: 64 v + 2 ones (z)
EXP_BIAS = -2.0       # constant softmax shift, cancels in normalization


def build_program(num_devices):
    nc = bacc.Bacc("TRN2", target_bir_lowering=False, debug=False,
                   num_devices=num_devices)

    def din(name, shape, dtype=F32):
        return nc.dram_tensor(name, shape, dtype, kind="ExternalInput").ap()

    # per-core activations
    seqT_d = din("seqT", [128, 4, NCH, 512], BF16)    # [p, sblk, ch, s]
    seqn_d = din("seqn", [S, D], BF16)                # row-gather source
    rowsel_d = din("rowsel", [128, 1], I16)
    # shared weights/tables
    statw_d = din("statw", [128, NCH, 2], BF16)       # [1, router_w]
    kvwk_d = din("kvwk", [128, NPR, 2, D], F8)
    kvwv_d = din("kvwv", [128, NPR, 2, D], F8)
    qw_d = din("qw", [128, NPR, 2, D], F8)
    ow_d = din("ow", [128, NPR, 2, D], F8)
    f1w_d = din("f1w", [128, NPR, 2, 2 * D], F8)
    f2w_d = din("f2w", [128, NPR, 2, D], F8)
    rotk_d = din("rotk", [128, S], F32)
    rotq_d = din("rotq", [S, 128], F32)
    ident_d = din("ident", [128, 128])
    tile16_d = din("tile16", [16, 128])
    iota1_d = din("iota1", [16, 128])
    vg_d = din("vg", [1, D])
    vbc_d = din("vbc", [128, NCH])
    qg_d = din("qg", [128, NCH])
    qb_d = din("qb", [128, NCH])
    f1b_d = din("f1b", [128, 16])
    f2b_d = din("f2b", [128, NCH])

    upd_d = nc.dram_tensor("updT", [128, NCH, NQ], BF16,
                           kind="ExternalOutput").ap()
    idx_d = nc.dram_tensor("idxsel", [16, 32], F32, kind="ExternalOutput").ap()

    tc_cm = tile.TileContext(nc)
    tc = tc_cm.__enter__()
    try:
        _emit(nc, tc, locals())
    finally:
        tc_cm.__exit__(None, None, None)
    nc.compile()
    return nc


def _emit(nc, tc, d):
    seqT_d, seqn_d, rowsel_d, statw_d = d["seqT_d"], d["seqn_d"], d["rowsel_d"], d["statw_d"]
    kvwk_d, kvwv_d, qw_d, ow_d = d["kvwk_d"], d["kvwv_d"], d["qw_d"], d["ow_d"]
    f1w_d, f2w_d, rotk_d, rotq_d = d["f1w_d"], d["f2w_d"], d["rotk_d"], d["rotq_d"]
    ident_d, tile16_d, iota1_d = d["ident_d"], d["tile16_d"], d["iota1_d"]
    vg_d, vbc_d, qg_d, qb_d = d["vg_d"], d["vbc_d"], d["qg_d"], d["qb_d"]
    f1b_d, f2b_d, upd_d, idx_d = d["f1b_d"], d["f2b_d"], d["upd_d"], d["idx_d"]
    from contextlib import ExitStack

    with ExitStack() as gctx:
        pc = gctx.enter_context(tc.tile_pool(name="const", bufs=1))
        pw = gctx.enter_context(tc.tile_pool(name="wts", bufs=1))
        rt = gctx.enter_context(tc.tile_pool(name="route", bufs=2))
        dram = gctx.enter_context(tc.tile_pool(name="dram", bufs=1, space="DRAM"))

        # ---- small consts ----
        identF = pc.tile([128, 128], F32)
        nc.sync.dma_start(identF[:], ident_d)
        ident = pc.tile([128, 128], BF16)
        nc.vector.tensor_copy(ident[:], identF[:])
        tile16 = pc.tile([16, 128], F32R)
        nc.sync.dma_start(tile16[:], tile16_d.bitcast(F32R))
        iota1 = pc.tile([16, 128], F32)
        nc.sync.dma_start(iota1[:], iota1_d)
        statw = pc.tile([128, NCH, 2], BF16)
        nc.sync.dma_start(statw[:], statw_d)
        rowsel = pc.tile([128, 1], I16)
        nc.sync.dma_start(rowsel[:], rowsel_d)
        vg = pc.tile([1, D], F32R)
        nc.sync.dma_start(vg[:], vg_d.bitcast(F32R))
        vbc = pc.tile([128, NCH], F32)
        nc.sync.dma_start(vbc[:], vbc_d)
        qg = pc.tile([128, NCH], F32)
        nc.sync.dma_start(qg[:], qg_d)
        qb = pc.tile([128, NCH], F32)
        nc.sync.dma_start(qb[:], qb_d)
        f1b = pc.tile([128, 16], F32)
        nc.sync.dma_start(f1b[:], f1b_d)
        f2b = pc.tile([128, NCH], F32)
        nc.sync.dma_start(f2b[:], f2b_d)
        rotk = pc.tile([128, S], F32)
        nc.sync.dma_start(rotk[:], rotk_d)
        onesP = pc.tile([128, 128], F32)
        nc.vector.memset(onesP[:], 1.0)
        onesPr = pc.tile([128, 128], F32R)
        nc.vector.tensor_copy(onesPr[:], onesP[:])
        ebias = pc.tile([128, 1], F32)
        nc.vector.memset(ebias[:], EXP_BIAS)
        idx16 = pc.tile([128, 32], I16)

        # ---- resident weights (6 big DMAs) ----
        kvwk = pw.tile([128, NPR, 2, D], F8)
        nc.sync.dma_start(kvwk[:], kvwk_d)
        kvwv = pw.tile([128, NPR, 2, D], F8)
        nc.sync.dma_start(kvwv[:], kvwv_d)
        qw = pw.tile([128, NPR, 2, D], F8)
        nc.sync.dma_start(qw[:], qw_d)
        f1w = pw.tile([128, NPR, 2, 2 * D], F8)
        nc.sync.dma_start(f1w[:], f1w_d)
        f2w = pw.tile([128, NPR, 2, D], F8)
        nc.sync.dma_start(f2w[:], f2w_d)
        ow = pw.tile([128, NPR, 2, D], F8)
        nc.sync.dma_start(ow[:], ow_d)

        scr_l = dram.tile([S], F32)
        scr_c = dram.tile([CAP], F32)
        scr_m = dram.tile([NQ], F32)

        prow = gctx.enter_context(tc.tile_pool(name="rows", bufs=1))
        lrow = prow.tile([1, S], F32)       # logits
        erow = prow.tile([1, S], F32R)      # exp(l - max)
        w_b = prow.tile([128, NQ], F32)

        pvln = gctx.enter_context(tc.tile_pool(name="vlnT", bufs=1))
        vlnT = pvln.tile([128, NPR, 2, S], F8)

        pseq_cm = tc.tile_pool(name="seqs", bufs=1)
        pseq = pseq_cm.__enter__()
        pab_cm = tc.tile_pool(name="abrows", bufs=1)
        pab = pab_cm.__enter__()
        seqT = pseq.tile([128, 4, NCH, 512], BF16)
        for nb in range(4):
            nc.sync.dma_start(seqT[:, nb, :, :], seqT_d[:, nb, :, :])
        rstd = pab.tile([1, S], F32R)
        nmr1 = pab.tile([1, S], F32R)       # -mu*rstd

        # ============ A1: stats ============
        with tc.tile_pool(name="a1rows", bufs=1) as pa1, \
             tc.tile_pool(name="sqp", bufs=3) as psq, \
             tc.tile_pool(name="ps_st", bufs=2, space="PSUM") as ps_st, \
             tc.tile_pool(name="ps_lg", bufs=2, space="PSUM") as ps_lg, \
             tc.tile_pool(name="ps_s2", bufs=2, space="PSUM") as ps_s2:
            srow = pa1.tile([1, S], F32)    # sums -> mu
            s2row = pa1.tile([1, S], F32)   # sumsq
            for nb in range(4):
                sl = slice(512 * nb, 512 * (nb + 1))
                stS = ps_st.tile([1, 512], F32, tag="stS")
                stL = ps_lg.tile([1, 512], F32, tag="stL")
                for ch in range(NCH):
                    nc.tensor.matmul(stS[:], statw[:, ch, 0:1],
                                     seqT[:, nb, ch, :],
                                     start=(ch == 0), stop=(ch == NCH - 1))
                for ch in range(NCH):
                    nc.tensor.matmul(stL[:], statw[:, ch, 1:2],
                                     seqT[:, nb, ch, :],
                                     start=(ch == 0), stop=(ch == NCH - 1))
                nc.vector.tensor_copy(srow[:, sl], stS[:])
                nc.vector.tensor_copy(lrow[:, sl], stL[:])
                st2 = ps_s2.tile([1, 512], F32, tag="st2")
                for ch in range(NCH):
                    sq = psq.tile([128, 512], BF16, tag="sq")
                    nc.scalar.activation(sq[:], seqT[:, nb, ch, :], AF.Square)
                    nc.tensor.matmul(st2[:], statw[:, ch, 0:1], sq[:],
                                     start=(ch == 0), stop=(ch == NCH - 1))
                nc.vector.tensor_copy(s2row[:, sl], st2[:])
            # logits -> DRAM for the routing reshapes
            nc.sync.dma_start(scr_l[:], lrow[:])
            # mu/var/rstd on partition 0
            nc.vector.tensor_scalar(srow[:], srow[:], 1.0 / D, None,
                                    op0=OP.mult)          # mu
            var = pa1.tile([1, S], F32)
            nc.vector.tensor_tensor(var[:], srow[:], srow[:], op=OP.mult)
            nc.vector.scalar_tensor_tensor(var[:], s2row[:], 1.0 / D, var[:],
                                           op0=OP.mult, op1=OP.subtract)
            nc.vector.tensor_scalar(var[:], var[:], 0.0, None, op0=OP.max)
            eps1 = prow.tile([1, 1], F32)
            nc.vector.memset(eps1[:], LN_EPS)
            sd = pa1.tile([1, S], F32)
            nc.scalar.activation(sd[:], var[:], AF.Sqrt, bias=eps1[:])
            with nc.allow_low_precision(reason="f32r bits"):
                nc.vector.reciprocal(rstd[:], sd[:])
            nc.vector.scalar_tensor_tensor(nmr1[:], srow[:], -1.0,
                                           rstd[:].bitcast(F32),
                                           op0=OP.mult, op1=OP.mult)
            # softmax row stats (all on partition 0)
            maxl = prow.tile([1, 1], F32)
            nc.vector.reduce_max(maxl[:], lrow[:], axis=AX.X)
            nmaxl = prow.tile([1, 1], F32)
            nc.vector.tensor_scalar(nmaxl[:], maxl[:], -1.0, None, op0=OP.mult)
            nc.scalar.activation(erow[:], lrow[:], AF.Exp, bias=nmaxl[:])
            zr = prow.tile([1, 1], F32)
            nc.vector.reduce_sum(zr[:], erow[:].bitcast(F32), axis=AX.X)
            rzr = prow.tile([1, 1], F32)
            nc.vector.reciprocal(rzr[:], zr[:])

        # ============ A2: routing (gpsimd) ============
        with tc.tile_pool(name="rt_ps", bufs=1, space="PSUM") as rt_ps:
            l128 = rt.tile([128, 16], F32)
            nc.sync.dma_start(l128[:], scr_l[:].rearrange("(f p) -> p f", p=128))
            l16 = rt.tile([16, 128], F32)
            nc.sync.dma_start(l16[:], scr_l[:].rearrange("(f p) -> p f", p=16))
            neg = rt.tile([128, 16], F32, tag="neg")
            nc.vector.memset(neg[:], -1e30)
            cur = l128
            nv = [2048, 1538, 1028]
            ks = [509, 509, 3]
            kout = None
            for i in range(3):
                kout = rt.tile([1, 2], F32, tag="kout")
                nc.gpsimd.kth_largest(kout[:], cur[:], n_per_lane=16, k=510,
                                      quantile=1.0 - (ks[i] + 0.5) / (nv[i] - 1.0))
                if i < 2:
                    tb = rt.tile([128, 2], F32, tag="tb")
                    nc.gpsimd.partition_broadcast(tb[:], kout[:])
                    gt = rt.tile([128, 16], U8, tag="gt")
                    nc.vector.tensor_scalar(gt[:], cur[:], tb[:, 1:2], None,
                                            op0=OP.is_gt)
                    nxt = rt.tile([128, 16], F32, tag="lm")
                    nc.vector.select(nxt[:], gt[:], neg[:], cur[:])
                    cur = nxt
            tb3 = rt.tile([128, 2], F32, tag="tb3")
            nc.gpsimd.partition_broadcast(tb3[:], kout[:])
            m16 = rt.tile([16, 128], F32)
            nc.vector.tensor_scalar(m16[:], l16[:], tb3[:16, 1:2], None,
                                    op0=OP.is_gt)
            mi = rt.tile([16, 128], F32)
            nc.vector.tensor_tensor(mi[:], iota1[:], m16[:], op=OP.mult)
            nc.vector.tensor_scalar_add(mi[:], mi[:], -1.0)
            compact = rt.tile([16, 64], F32)
            nfound = rt.tile([1, 1], U32)
            nc.gpsimd.sparse_gather(compact[:], mi[:], num_found=nfound[:])
            nc.sync.dma_start(scr_c[:], compact[:])
            rows16 = rt.tile([128, 1, 64], F32)
            nc.gpsimd.dma_gather(rows16[:],
                                 scr_c[:].rearrange("(r c) -> r c", c=64),
                                 rowsel[:], num_idxs=8, num_idxs_reg=8,
                                 elem_size=64)
            nc.sync.dma_start(scr_m[:], rows16[0:8, 0, :])
            nc.sync.dma_start(idx_d, scr_m[:].rearrange("(f p) -> p f", p=16))
            mc_r = rt.tile([16, 32], F32R)
            nc.sync.dma_start(mc_r[:],
                              scr_m[:].rearrange("(f p) -> p f", p=16)
                              .bitcast(F32R))
            idxp = rt_ps.tile([128, 32], F32, tag="idxp")
            nc.tensor.matmul(idxp[:], tile16[:], mc_r[:], start=True, stop=True)
            nc.vector.tensor_copy(idx16[:], idxp[:])

        # ============ A3: vlnT (fp8) ============
        with tc.tile_pool(name="vtmp", bufs=3) as pvt, \
             tc.tile_pool(name="ps_bc", bufs=4, space="PSUM") as ps_bc:
            for ch in range(NCH):
                cs = slice(128 * ch, 128 * (ch + 1))
                for nb in range(4):
                    sl = slice(512 * nb, 512 * (nb + 1))
                    rbp = ps_bc.tile([128, 512], F32, tag="rb")
                    nc.tensor.matmul(rbp[:], vg[:, cs],
                                     rstd[:, sl],
                                     start=True, stop=True)
                    ab1 = ps_bc.tile([128, 512], F32, tag="ab")
                    nc.tensor.matmul(ab1[:], vg[:, cs],
                                     nmr1[:, sl],
                                     start=True, stop=True)
                    vt = pvt.tile([128, 512], F32, tag="vt")
                    nc.vector.tensor_tensor(vt[:], seqT[:, nb, ch, :], rbp[:],
                                            op=OP.mult)
                    nc.vector.scalar_tensor_tensor(vlnT[:, ch // 2, ch % 2, sl],
                                                   vt[:], vbc[:, ch:ch + 1],
                                                   ab1[:], op0=OP.add,
                                                   op1=OP.add)

        pab_cm.__exit__(None, None, None)
        pseq_cm.__exit__(None, None, None)

        # ============ B1: kT (bf16) + V (fp8) ============
        pkt = gctx.enter_context(tc.tile_pool(name="ktp", bufs=1))
        kt = pkt.tile([128, NJCH, S], BF16)
        pv2 = gctx.enter_context(tc.tile_pool(name="v2p", bufs=1))
        v2 = pv2.tile([128, NSCH // 2, 2, H, VE], F8)
        nc.vector.memset(v2[:, :, :, :, HD:], 1.0)
        with tc.tile_pool(name="ps_kv", bufs=2, space="PSUM") as ps_kv, \
             tc.tile_pool(name="ps_v", bufs=2, space="PSUM") as ps_v:
            for jch in range(NJCH):
                js = slice(128 * jch, 128 * (jch + 1))
                for nb in range(4):
                    sl = slice(512 * nb, 512 * (nb + 1))
                    kp = ps_kv.tile([128, 512], F32, tag="kp")
                    for c in range(NPR):
                        nc.tensor.matmul(kp[:], kvwk[:, c, :, js],
                                         vlnT[:, c, :, sl],
                                         start=(c == 0), stop=(c == NPR - 1),
                                         perf_mode=DR)
                    nc.vector.tensor_tensor(kt[:, jch, sl], kp[:],
                                            rotk[:, sl], op=OP.mult)
            for sch in range(NSCH):
                ss = slice(128 * sch, 128 * (sch + 1))
                for hb in range(2):
                    hs = slice(512 * hb, 512 * (hb + 1))
                    vp = ps_v.tile([128, 512], F32, tag="vp")
                    for c in range(NPR):
                        nc.tensor.matmul(vp[:], vlnT[:, c, :, ss],
                                         kvwv[:, c, :, hs],
                                         start=(c == 0), stop=(c == NPR - 1),
                                         perf_mode=DR)
                    nc.scalar.copy(
                        v2[:, sch // 2, sch % 2, 8 * hb:8 * (hb + 1), 0:HD],
                        vp[:].rearrange("p (h v) -> p h v", v=HD))

        # ============ A4: q path ============
        pq = gctx.enter_context(tc.tile_pool(name="qp", bufs=1))
        qlnT = pq.tile([128, NPR, 2, NQ], F8)
        qTr = pq.tile([128, NJCH, NQ], BF16)
        with tc.tile_pool(name="qgat", bufs=1) as pqg, \
             tc.tile_pool(name="ps_q", bufs=2, space="PSUM") as ps_q, \
             tc.tile_pool(name="ps_tr", bufs=2, space="PSUM") as ps_tr:
            qseq = pqg.tile([128, 4, D], BF16)
            nc.gpsimd.dma_gather(qseq[:], seqn_d, idx16[:], num_idxs=NQ,
                                 num_idxs_reg=NQ, elem_size=D)
            rotq_g = pqg.tile([128, 4, 128], F32)
            nc.gpsimd.dma_gather(rotq_g[:], rotq_d, idx16[:], num_idxs=NQ,
                                 num_idxs_reg=NQ, elem_size=128)
            bst = pqg.tile([128, 4, 2, 6], F32)
            for g in range(4):
                for hh in range(2):
                    nc.vector.bn_stats(bst[:, g, hh, :],
                                       qseq[:, g, 512 * hh:512 * (hh + 1)])
            mv = pqg.tile([128, 4, 2], F32)
            for g in range(4):
                nc.vector.bn_aggr(mv[:, g, :], bst[:, g, :, :])
            sd_s = pqg.tile([128, 4], F32)
            eps128 = pqg.tile([128, 1], F32)
            nc.vector.memset(eps128[:], LN_EPS)
            nc.scalar.activation(sd_s[:], mv[:, :, 1], AF.Sqrt, bias=eps128[:])
            rstd_s = pqg.tile([128, 4], F32)
            nc.vector.reciprocal(rstd_s[:], sd_s[:])
            qn = pqg.tile([128, 4, D], BF16)
            for g in range(4):
                nc.vector.tensor_scalar(qn[:, g, :], qseq[:, g, :],
                                        mv[:, g, 0:1], rstd_s[:, g:g + 1],
                                        op0=OP.subtract, op1=OP.mult)
            for g in range(4):
                for ch in range(NCH):
                    tp = ps_tr.tile([128, 128], BF16, tag="tp")
                    nc.tensor.transpose(tp[:], qn[:, g, 128 * ch:128 * (ch + 1)],
                                        ident[:])
                    nc.vector.tensor_scalar(
                        qlnT[:, ch // 2, ch % 2, 128 * g:128 * (g + 1)],
                        tp[:], qg[:, ch:ch + 1], qb[:, ch:ch + 1],
                        op0=OP.mult, op1=OP.add)
            rotqT = pqg.tile([128, NQ], F32)
            for g in range(4):
                tpf = ps_tr.tile([128, 128], F32, tag="tpf")
                nc.tensor.transpose(tpf[:], rotq_g[:, g, :],
                                    identF[:])
                nc.vector.tensor_copy(rotqT[:, 128 * g:128 * (g + 1)], tpf[:])
            for jch in range(NJCH):
                js = slice(128 * jch, 128 * (jch + 1))
                qp_ps = ps_q.tile([128, NQ], F32, tag="qpp")
                for c in range(NPR):
                    nc.tensor.matmul(qp_ps[:], qw[:, c, :, js],
                                     qlnT[:, c, :, :],
                                     start=(c == 0), stop=(c == NPR - 1),
                                     perf_mode=DR)
                nc.vector.tensor_tensor(qTr[:, jch, :], qp_ps[:], rotqT[:],
                                        op=OP.mult)
            # w_b: per-query router weight broadcast tile
            rz16 = pqg.tile([1, 16], F32R)
            nc.vector.tensor_scalar(rz16[:], onesP[0:1, 0:16],
                                    rzr[:, 0:1], None, op0=OP.mult)
            e16f = pqg.tile([16, S], F32)
            with tc.tile_pool(name="ps_w", bufs=1, space="PSUM") as ps_w:
                for nb in range(4):
                    sl = slice(512 * nb, 512 * (nb + 1))
                    ep = ps_w.tile([16, 512], F32, tag="ep")
                    nc.tensor.matmul(ep[:], rz16[:],
                                     erow[0:1, sl],
                                     start=True, stop=True)
                    nc.vector.tensor_copy(e16f[:, sl], ep[:])
                esel = pqg.tile([16, NQ], F32)
                nc.gpsimd.ap_gather(esel[:], e16f[:], idx16[0:16, :],
                                    channels=16, num_elems=S, d=1, num_idxs=NQ)
                w_row = pqg.tile([1, NQ], F32R)
                nc.vector.tensor_scalar(w_row[:], esel[0:1, :], 1.0, None,
                                        op0=OP.mult)
                wp = ps_w.tile([128, NQ], F32, tag="wp")
                nc.tensor.matmul(wp[:], onesPr[0:1, :], w_row[:],
                                 start=True, stop=True)
                nc.vector.tensor_copy(w_b[:], wp[:])

        # ============ B2: attention ============
        pattn = gctx.enter_context(tc.tile_pool(name="attn", bufs=1))
        attn = pattn.tile([128, NJCH // 2, 2, NQ], F8)
        with tc.tile_pool(name="prb", bufs=2) as ppr, \
             tc.tile_pool(name="rzb", bufs=2) as prz, \
             tc.tile_pool(name="oddh", bufs=2) as podd, \
             tc.tile_pool(name="ps_sa", bufs=2, space="PSUM") as ps_sa, \
             tc.tile_pool(name="ps_sb", bufs=2, space="PSUM") as ps_sb, \
             tc.tile_pool(name="ps_at", bufs=2, space="PSUM") as ps_at, \
             tc.tile_pool(name="ps_rp", bufs=1, space="PSUM") as ps_rp:
            for jch in range(NJCH):
                at0 = ps_at.tile([VE, NQ], F32, tag="at")
                at1 = ps_at.tile([VE, NQ], F32, tag="at")
                for cp in range(NSCH // 2):
                    pr0 = ppr.tile([128, 2, NQ], F8, tag="pr0")
                    pr1 = ppr.tile([128, 2, NQ], F8, tag="pr1")
                    for k in range(2):
                        sch = 2 * cp + k
                        ksl = kt[:, jch, 128 * sch:128 * (sch + 1)]
                        sc0 = ps_sa.tile([128, NQ], F32, tag="sc0")
                        sc1 = ps_sb.tile([128, NQ], F32, tag="sc1")
                        nc.tensor.matmul(sc0[:], ksl[0:64, :],
                                         qTr[0:64, jch, :], start=True,
                                         stop=True, tile_position=(0, 0))
                        nc.tensor.matmul(sc1[:], ksl[64:128, :],
                                         qTr[64:128, jch, :], start=True,
                                         stop=True, tile_position=(64, 0))
                        nc.scalar.activation(pr0[:, k, :], sc0[:], AF.Exp,
                                             bias=ebias[:])
                        nc.scalar.activation(pr1[:, k, :], sc1[:], AF.Exp,
                                             bias=ebias[:])
                    nc.tensor.matmul(at0[:], v2[:, cp, :, 2 * jch, :], pr0[:],
                                     start=(cp == 0), stop=(cp == NSCH // 2 - 1),
                                     perf_mode=DR)
                    nc.tensor.matmul(at1[:], v2[:, cp, :, 2 * jch + 1, :], pr1[:],
                                     start=(cp == 0), stop=(cp == NSCH // 2 - 1),
                                     perf_mode=DR)
                for hh, at in ((0, at0), (1, at1)):
                    rz = prz.tile([128, NQ], F32R, tag="rz")
                    with nc.allow_low_precision(reason="f32r bits"):
                        nc.vector.reciprocal(rz[HD:HD + 1, :], at[HD:HD + 1, :])
                    rp = ps_rp.tile([64, NQ], F32, tag="rp")
                    nc.tensor.matmul(rp[:], onesPr[HD:HD + 1, 0:64],
                                     rz[HD:HD + 1, :],
                                     start=True, stop=True,
                                     tile_position=(64, 0))
                    rsb = prz.tile([64, NQ], F32, tag="rsb")
                    nc.vector.tensor_copy(rsb[:], rp[:])
                    if hh == 0:
                        nc.vector.tensor_tensor(attn[0:64, jch // 2, jch % 2, :],
                                                at[0:HD, :], rsb[:], op=OP.mult)
                    else:
                        tmp = podd.tile([64, NQ], F8, tag="tmp")
                        nc.vector.tensor_tensor(tmp[:], at[0:HD, :], rsb[:],
                                                op=OP.mult)
                        nc.sync.dma_start(attn[64:128, jch // 2, jch % 2, :],
                                          tmp[:])

        # ============ C: FFN + out-proj + update ============
        with tc.tile_pool(name="sTp", bufs=1) as psT, \
             tc.tile_pool(name="silu", bufs=3) as psl, \
             tc.tile_pool(name="updp", bufs=2) as pup, \
             tc.tile_pool(name="ps_c", bufs=2, space="PSUM") as ps_c:
            sT = psT.tile([128, NPR, 2, NQ], F8)
            for j2 in range(NCH):
                xs = slice(128 * j2, 128 * (j2 + 1))
                gs = slice(D + 128 * j2, D + 128 * (j2 + 1))
                xp = ps_c.tile([128, NQ], F32, tag="xp")
                for c in range(NPR):
                    nc.tensor.matmul(xp[:], f1w[:, c, :, xs], qlnT[:, c, :, :],
                                     start=(c == 0), stop=(c == NPR - 1),
                                     perf_mode=DR)
                gp = ps_c.tile([128, NQ], F32, tag="gp")
                for c in range(NPR):
                    nc.tensor.matmul(gp[:], f1w[:, c, :, gs], qlnT[:, c, :, :],
                                     start=(c == 0), stop=(c == NPR - 1),
                                     perf_mode=DR)
                x1 = psl.tile([128, NQ], F32, tag="x1")
                nc.vector.tensor_scalar(x1[:], xp[:], f1b[:, j2:j2 + 1],
                                        None, op0=OP.add)
                gb = psl.tile([128, NQ], F32, tag="gb")
                nc.vector.tensor_scalar(gb[:], gp[:], f1b[:, 8 + j2:9 + j2],
                                        None, op0=OP.add)
                sl_t = psl.tile([128, NQ], F32, tag="slt")
                nc.scalar.activation(sl_t[:], gp[:], AF.Sigmoid,
                                     bias=f1b[:, 8 + j2:9 + j2])
                nc.vector.tensor_tensor(sl_t[:], sl_t[:], gb[:], op=OP.mult)
                nc.vector.tensor_tensor(sT[:, j2 // 2, j2 % 2, :], sl_t[:],
                                        x1[:], op=OP.mult)

            for ich in range(NCH):
                is_ = slice(128 * ich, 128 * (ich + 1))
                op_ = ps_c.tile([128, NQ], F32, tag="op")
                for c in range(NPR):
                    nc.tensor.matmul(op_[:], ow[:, c, :, is_],
                                     attn[:, c, :, :],
                                     start=(c == 0), stop=(c == NPR - 1),
                                     perf_mode=DR)
                fp = ps_c.tile([128, NQ], F32, tag="fp")
                for c in range(NPR):
                    nc.tensor.matmul(fp[:], f2w[:, c, :, is_], sT[:, c, :, :],
                                     start=(c == 0), stop=(c == NPR - 1),
                                     perf_mode=DR)
                fs = pup.tile([128, NQ], F32, tag="fs")
                nc.vector.tensor_scalar(fs[:], fp[:], f2b[:, ich:ich + 1],
                                        None, op0=OP.add)
                nc.vector.tensor_tensor(fs[:], op_[:], fs[:], op=OP.add)
                ut = pup.tile([128, NQ], BF16, tag="ut")
                nc.vector.tensor_tensor(ut[:], fs[:], w_b[:], op=OP.mult)
                nc.sync.dma_start(upd_d[:, ich, :], ut[:])


def _rope_table():
    freqs = np.exp(np.linspace(0.0, -1.0, HD // 2) * np.log(ROPE_BASE))
    pos = np.arange(S, dtype=np.float64)
    ang = pos[:, None] * freqs[None, :].astype(np.float64)
    rot = np.concatenate([np.sin(ang), np.cos(ang)], axis=1)
    return rot.astype(np.float32)


def _make_host_tables():
    rot = _rope_table()                          # (S, 64)
    rotk = np.empty((128, S), np.float32)
    for r in range(128):
        rotk[r] = rot[:, r % HD]
    rotq = np.ascontiguousarray(
        np.concatenate([rot, rot], axis=1) / np.float32(8.0))
    tile16 = np.zeros((16, 128), np.float32)
    for m in range(128):
        tile16[m % 16, m] = 1.0
    iota1 = np.empty((16, 128), np.float32)
    for p in range(16):
        for f in range(128):
            iota1[p, f] = f * 16 + p + 1
    ident = np.eye(128, dtype=np.float32)
    return rotk, rotq, tile16, iota1, ident


def _pair_layout(wT, ncols):
    # wT: [D_in, ncols] -> [128, NPR, 2, ncols] fp8 (DoubleRow lhsT layout)
    w = np.asarray(wT, np.float32).reshape(NPR, 2, 128, ncols)
    return np.ascontiguousarray(w.transpose(2, 0, 1, 3)).astype(NP_F8)


def make_in_maps(inputs, n_cores=8):
    seq = np.asarray(inputs["seq"], np.float32)
    rotk, rotq, tile16, iota1, ident = _make_host_tables()
    kvT = np.ascontiguousarray(np.asarray(inputs["kv_w"], np.float32).T)
    statw = np.empty((128, NCH, 2), NP_BF16)
    rw = np.asarray(inputs["router_w"], np.float32).reshape(NCH, 128)
    for c in range(NCH):
        statw[:, c, 0] = 1.0
        statw[:, c, 1] = rw[c].astype(NP_BF16)
    shared = {
        "statw": statw,
        "kvwk": _pair_layout(kvT[:, :D], D),
        "kvwv": _pair_layout(kvT[:, D:], D),
        "qw": _pair_layout(np.asarray(inputs["q_w"], np.float32).T, D),
        "ow": _pair_layout(np.asarray(inputs["out_w"], np.float32).T, D),
        "f1w": _pair_layout(np.asarray(inputs["fc1_w"], np.float32).T, 2 * D),
        "f2w": _pair_layout(np.asarray(inputs["fc2_w"], np.float32).T, D),
        "rotk": rotk, "rotq": rotq, "ident": ident,
        "tile16": tile16, "iota1": iota1,
        "vg": np.asarray(inputs["vln_g"], np.float32).reshape(1, D).copy(),
        "vbc": np.ascontiguousarray(
            np.asarray(inputs["vln_b"], np.float32).reshape(NCH, 128).T),
        "qg": np.ascontiguousarray(
            np.asarray(inputs["qln_g"], np.float32).reshape(NCH, 128).T),
        "qb": np.ascontiguousarray(
            np.asarray(inputs["qln_b"], np.float32).reshape(NCH, 128).T),
        "f1b": np.ascontiguousarray(
            np.asarray(inputs["fc1_b"], np.float32).reshape(16, 128).T),
        "f2b": np.ascontiguousarray(
            np.asarray(inputs["fc2_b"], np.float32).reshape(NCH, 128).T),
    }
    in_maps = []
    for c in range(n_cores):
        b, p = c // 2, c % 2
        rowsel = np.empty((128, 1), np.int16)
        for P in range(128):
            rowsel[P, 0] = 8 * p + (P % 16) % 8
        m = dict(shared)
        # seqT host layout: [p, sblk, ch, s] = seq[b, 512*sblk+s, 128*ch+p]
        st = seq[b].reshape(4, 512, NCH, 128)          # [sblk, s, ch, p]
        m["seqT"] = np.ascontiguousarray(
            st.transpose(3, 0, 2, 1)).astype(NP_BF16)
        m["seqn"] = np.ascontiguousarray(seq[b]).astype(NP_BF16)
        m["rowsel"] = rowsel
        in_maps.append(m)
    return in_maps


def assemble_output(seq, results):
    out = np.array(seq, np.float32, copy=True)
    for c, r in enumerate(results):
        b = c // 2
        upd = np.asarray(r["updT"], dtype=NP_BF16).astype(np.float32)
        upd = upd.transpose(1, 0, 2).reshape(D, NQ)    # [i, q]
        idxw = np.asarray(r["idxsel"])
        idx = np.empty(NQ, np.int64)
        for j in range(NQ):
            idx[j] = int(idxw[j % 16, j // 16])
        out[b, idx, :] += upd.T
    return out


_PROGRAM = None


def kernel(**inputs):
    global _PROGRAM
    seq = np.asarray(inputs["seq"], np.float32)
    if _PROGRAM is None:
        _PROGRAM = build_program(8)
    in_maps = make_in_maps(inputs, 8)
    res = run_bass_kernel_spmd(_PROGRAM, in_maps, list(range(8)))
    return assemble_output(seq, res.results)
